# revision 1
# baseline (speedup 1.0000x reference)
"""MultiHeadAttention TRN2 Bass kernel (8 NeuronCores).

Sharding: core c = (batch b = c//2, query-half = c%2). Each core computes
K/V for its full batch (2048 keys) and attention + output projection + LN
for its 1024 query rows. No collectives; host gathers per-core outputs.

Device math (all matmuls in float32r = full-rate fp32, rel err ~2e-4):
  QhT[hd, q]  = wq[d, hd].T @ qT[d, q]          (per 4-head phase)
  KhT[hd, m]  = wk[d, hd].T @ qT[d, m]
  Vaug[m, 65] = qT[d, m].T @ wv[d, 65-packed]   (+ ones column)
  S^T[m, q]   = KhT[dk, m].T @ QhT[dk, q]       (K=64, head pairs packed
                                                 via tile_position rows)
  E = exp(S^T / 32)                              (ACT, PSUM->SBUF fp32r)
  OT[65, q]   = sum_m Vaug[m,65].T @ E[m, q]    (row 64 = softmax denom)
  CT[c, q]    = OT[0:64] * (1/denom)            (K=1 ones-matmul bcast +
                                                 reciprocal_approx_fast)
  Y[q, o]     = CT[c, q].T @ pwT[c, o] + (q_res + proj_b)   then LayerNorm
"""
import numpy as np

import concourse.bass as bass
import concourse.mybir as mybir
import concourse.tile as tile
from concourse import bacc
from concourse.bass_utils import run_bass_kernel_spmd

F32 = mybir.dt.float32
F32R = mybir.dt.float32r
AF = mybir.ActivationFunctionType
ALU = mybir.AluOpType
AX = mybir.AxisListType

B, L, D = 4, 2048, 1024
H, DK = 16, 64
HALF = 1024            # query rows per core
TEMPER = 32.0          # sqrt(d_model)
PHASES = 4
HP = H // PHASES       # 4 heads per phase
PAIRS = HP // 2        # 2 head-pairs per phase
MT = L // 128          # 16 m-tiles
LN_EPS = 1e-3

_CACHE = {}


def build(iters=1):
    nc = bacc.Bacc(None, target_bir_lowering=False)
    qt_d = nc.dram_tensor("qt", [D, L], F32R, kind="ExternalInput")
    qres_d = nc.dram_tensor("qres", [HALF, D], F32, kind="ExternalInput")
    wq_d = nc.dram_tensor("wq", [D, H * DK], F32R, kind="ExternalInput")
    wk_d = nc.dram_tensor("wk", [D, H * DK], F32R, kind="ExternalInput")
    wv_d = nc.dram_tensor("wv", [D, H * 65], F32R, kind="ExternalInput")
    pw_d = nc.dram_tensor("pw", [D, D], F32R, kind="ExternalInput")
    lna_d = nc.dram_tensor("lna", [1, D], F32, kind="ExternalInput")
    lnb_d = nc.dram_tensor("lnb", [1, D], F32, kind="ExternalInput")
    ones_d = nc.dram_tensor("ones64", [1, 64], F32R, kind="ExternalInput")
    out_d = nc.dram_tensor("out", [HALF, D], F32, kind="ExternalOutput")

    with tile.TileContext(nc) as tc:
        with (
            tc.tile_pool(name="p1", bufs=1) as p1,
            tc.tile_pool(name="p2", bufs=2) as p2,
            tc.tile_pool(name="psA", bufs=4, space="PSUM") as psA,
            tc.tile_pool(name="psS", bufs=2, space="PSUM") as psS,
        ):
            # ---- one-time constants ----
            ones_t = p1.tile([128, 64], F32R, name="ones_t")
            nc.sync.dma_start(ones_t[64:65, :], ones_d[:])
            ones_sb = p1.tile([128, 16], F32R, name="ones_sb")
            nc.sync.dma_start(ones_sb[:], ones_d[:, 0:16].to_broadcast([128, 16]))
            lna_t = p1.tile([128, D], F32, name="lna_t")
            nc.sync.dma_start(lna_t[:], lna_d[:].to_broadcast([128, D]))
            lnb_t = p1.tile([128, D], F32, name="lnb_t")
            nc.sync.dma_start(lnb_t[:], lnb_d[:].to_broadcast([128, D]))

            ct_t = p1.tile([128, H // 2, HALF], F32R, name="ct_t")

            for it in range(iters):
                for p in range(PHASES):
                    c0 = p * HP * DK
                    wq_t = p1.tile([128, 8, HP * DK], F32R, name=f"it{it}_wq_{p}", tag="wq")
                    nc.sync.dma_start(
                        wq_t[:],
                        wq_d[:, c0:c0 + HP * DK].rearrange("(dj pp) f -> pp dj f", pp=128),
                    )
                    wk_t = p1.tile([128, 8, HP * DK], F32R, name=f"it{it}_wk_{p}", tag="wk")
                    nc.sync.dma_start(
                        wk_t[:],
                        wk_d[:, c0:c0 + HP * DK].rearrange("(dj pp) f -> pp dj f", pp=128),
                    )
                    v0 = p * HP * 65
                    wv_t = p1.tile([128, 8, HP * 65], F32R, name=f"it{it}_wv_{p}", tag="wv")
                    nc.sync.dma_start(
                        wv_t[:],
                        wv_d[:, v0:v0 + HP * 65].rearrange("(dj pp) f -> pp dj f", pp=128),
                    )

                    qht_t = p2.tile([128, PAIRS, HALF], F32R, name=f"it{it}_qht_{p}", tag="qht")
                    kht_t = p2.tile([128, PAIRS, L], F32R, name=f"it{it}_kht_{p}", tag="kht")
                    vaug_t = p2.tile([128, MT, HP * 65], F32R, name=f"it{it}_vaug_{p}", tag="vaug")

                    # ---- QKV projections, streaming qT in 512-col blocks ----
                    for mc in range(L // 512):
                        qt_t = p2.tile([128, 8, 512], F32R, name=f"it{it}_qt_{p}_{mc}", tag="qt")
                        nc.sync.dma_start(
                            qt_t[:],
                            qt_d[:, mc * 512:(mc + 1) * 512].rearrange(
                                "(dj pp) m -> pp dj m", pp=128
                            ),
                        )
                        # K (and Q for the first half of columns): 2+2 psum groups
                        for mt in range(PAIRS):
                            kps = psA.tile([128, 512], F32, name=f"it{it}_kps_{p}_{mc}_{mt}",
                                           tag="acc")
                            for dj in range(8):
                                nc.tensor.matmul(
                                    kps[:],
                                    wk_t[:, dj, mt * 128:(mt + 1) * 128],
                                    qt_t[:, dj, :],
                                    start=(dj == 0), stop=(dj == 7),
                                )
                            nc.vector.tensor_copy(
                                kht_t[:, mt, mc * 512:(mc + 1) * 512], kps[:]
                            )
                        if mc < HALF // 512:
                            for mt in range(PAIRS):
                                qps = psA.tile([128, 512], F32, name=f"it{it}_qps_{p}_{mc}_{mt}",
                                               tag="acc")
                                for dj in range(8):
                                    nc.tensor.matmul(
                                        qps[:],
                                        wq_t[:, dj, mt * 128:(mt + 1) * 128],
                                        qt_t[:, dj, :],
                                        start=(dj == 0), stop=(dj == 7),
                                    )
                                nc.vector.tensor_copy(
                                    qht_t[:, mt, mc * 512:(mc + 1) * 512], qps[:]
                                )
                        # V: 4 m-subtiles of 128, N = HP*65 = 260
                        for ms in range(4):
                            mi = mc * 4 + ms
                            vps = psA.tile([128, HP * 65], F32, name=f"it{it}_vps_{p}_{mi}",
                                           tag="acc")
                            for dj in range(8):
                                nc.tensor.matmul(
                                    vps[:],
                                    qt_t[:, dj, ms * 128:(ms + 1) * 128],
                                    wv_t[:, dj, :],
                                    start=(dj == 0), stop=(dj == 7),
                                )
                            nc.vector.tensor_copy(vaug_t[:, mi, :], vps[:])
                    # ones columns of V_aug
                    for hl in range(HP):
                        nc.vector.tensor_copy(
                            vaug_t[:, :, hl * 65 + 64], ones_sb[:, 0:MT]
                        )

                    # ---- attention ----
                    for a in range(PAIRS):
                        cj = p * PAIRS + a
                        for qc in range(HALF // 512):
                            qs = slice(qc * 512, (qc + 1) * 512)
                            ot = {}
                            for par in range(2):
                                ot[par] = psA.tile([65, 512], F32,
                                                   name=f"it{it}_ot_{p}_{a}_{qc}_{par}",
                                                   tag="acc")
                            for mi in range(MT):
                                ms_ = slice(mi * 128, (mi + 1) * 128)
                                sp = psS.tile([128, 1024], F32,
                                              name=f"it{it}_s_{p}_{a}_{qc}_{mi}",
                                              tag="score")
                                for par in range(2):
                                    nc.tensor.matmul(
                                        sp[:, 512 * par:512 * (par + 1)],
                                        kht_t[64 * par:64 * (par + 1), a, ms_],
                                        qht_t[64 * par:64 * (par + 1), a, qs],
                                        start=True, stop=True,
                                        tile_position=(64 * par, 0),
                                    )
                                ex = p2.tile([128, 1024], F32R,
                                             name=f"it{it}_e_{p}_{a}_{qc}_{mi}",
                                             tag="exp")
                                nc.scalar.activation(ex[:], sp[:], AF.Exp,
                                                     scale=1.0 / TEMPER)
                                for par in range(2):
                                    hl = 2 * a + par
                                    nc.tensor.matmul(
                                        ot[par][:],
                                        vaug_t[:, mi, hl * 65:(hl + 1) * 65],
                                        ex[:, 512 * par:512 * (par + 1)],
                                        start=(mi == 0), stop=(mi == MT - 1),
                                    )
                            for par in range(2):
                                den = p1.tile([128, 512], F32R,
                                              name=f"it{it}_den_{p}_{a}_{qc}_{par}", tag="den")
                                nc.vector.tensor_copy(den[64:65, :], ot[par][64:65, :])
                                bc = psA.tile([64, 512], F32,
                                              name=f"it{it}_bc_{p}_{a}_{qc}_{par}", tag="acc")
                                nc.tensor.matmul(bc[:], ones_t[64:65, :],
                                                 den[64:65, :], start=True, stop=True)
                                rec = p1.tile([64, 512], F32,
                                              name=f"it{it}_rec_{p}_{a}_{qc}_{par}", tag="rec")
                                nc.vector.reciprocal_approx_fast(rec[:], bc[:])
                                if par == 0:
                                    nc.vector.tensor_mul(
                                        ct_t[0:64, cj, qs], ot[par][0:64, :], rec[:]
                                    )
                                else:
                                    stg = p1.tile([64, 512], F32R,
                                                  name=f"it{it}_stg_{p}_{a}_{qc}", tag="stg")
                                    nc.vector.tensor_mul(stg[:], ot[par][0:64, :], rec[:])
                                    nc.sync.dma_start(ct_t[64:128, cj, qs], stg[:])

                # ---- output projection + residual + layernorm ----
                pw_t = {}
                for oc in range(2):
                    pw_t[oc] = p2.tile([128, 8, 512], F32R,
                                       name=f"it{it}_pwt_{oc}", tag="kht")
                    nc.sync.dma_start(
                        pw_t[oc][:],
                        pw_d[:, oc * 512:(oc + 1) * 512].rearrange(
                            "(dj pp) f -> pp dj f", pp=128
                        ),
                    )
                for qtb in range(4):
                    yts = {}
                    for qt in range(2):
                        yts[qt] = p2.tile([128, D], F32,
                                          name=f"it{it}_yt_{qtb}_{qt}", tag="y")
                        qti0 = qtb * 2 + qt
                        nc.sync.dma_start(
                            yts[qt][:], qres_d[qti0 * 128:(qti0 + 1) * 128, :])
                    for oc in range(2):
                        ypss = {}
                        for cjj in range(H // 2):
                            for qt in range(2):
                                qti = qtb * 2 + qt
                                if cjj == 0:
                                    ypss[qt] = psA.tile(
                                        [128, 512], F32,
                                        name=f"it{it}_y_{qtb}_{oc}_{qt}", tag="acc")
                                nc.tensor.matmul(
                                    ypss[qt][:],
                                    ct_t[:, cjj, qti * 128:(qti + 1) * 128],
                                    pw_t[oc][:, cjj, :],
                                    start=(cjj == 0), stop=(cjj == H // 2 - 1),
                                )
                        for qt in range(2):
                            nc.vector.tensor_add(
                                yts[qt][:, oc * 512:(oc + 1) * 512],
                                yts[qt][:, oc * 512:(oc + 1) * 512],
                                ypss[qt][:],
                            )
                    for qt in range(2):
                        qti = qtb * 2 + qt
                        y_t = yts[qt]
                        # layernorm: mu, sigma (ddof=1), (y-mu)/(sigma+eps)*a+b
                        s = p1.tile([128, 1], F32, name=f"it{it}_s_{qti}", tag="ln_s")
                        nc.vector.reduce_sum(s[:], y_t[:], axis=AX.X)
                        negmean = p1.tile([128, 1], F32, name=f"it{it}_nm_{qti}", tag="ln_nm")
                        nc.vector.tensor_scalar_mul(negmean[:], s[:], -1.0 / D)
                        mean = p1.tile([128, 1], F32, name=f"it{it}_m_{qti}", tag="ln_m")
                        nc.vector.tensor_scalar_mul(mean[:], s[:], 1.0 / D)
                        ss = p1.tile([128, 1], F32, name=f"it{it}_ss_{qti}", tag="ln_ss")
                        ss2 = p1.tile([128, 1], F32, name=f"it{it}_ss2_{qti}", tag="ln_ss2")
                        for oc in range(2):
                            sq = psS.tile([128, 512], F32, name=f"it{it}_sq_{qti}_{oc}",
                                          tag="score")
                            nc.scalar.activation(
                                sq[:], y_t[:, oc * 512:(oc + 1) * 512], AF.Square,
                                bias=negmean[:],
                                accum_out=(ss[:] if oc == 0 else ss2[:]),
                            )
                        nc.vector.tensor_add(ss[:], ss[:], ss2[:])
                        sigma = p1.tile([128, 1], F32, name=f"it{it}_sg_{qti}", tag="ln_sg")
                        nc.scalar.activation(sigma[:], ss[:], AF.Sqrt,
                                             scale=1.0 / (D - 1))
                        var = p1.tile([128, 1], F32, name=f"it{it}_var_{qti}", tag="ln_var")
                        nc.vector.tensor_scalar_mul(var[:], ss[:], 1.0 / (D - 1))
                        rs = p1.tile([128, 1], F32, name=f"it{it}_rs_{qti}", tag="ln_rs")
                        nc.vector.reciprocal(rs[:], sigma[:])
                        t1 = p1.tile([128, 1], F32, name=f"it{it}_t1_{qti}", tag="ln_t1")
                        nc.vector.tensor_mul(t1[:], var[:], rs[:])
                        nc.vector.tensor_add(t1[:], t1[:], sigma[:])
                        dd = p1.tile([128, 1], F32, name=f"it{it}_dd_{qti}", tag="ln_dd")
                        nc.vector.tensor_scalar(dd[:], t1[:], 0.5, LN_EPS,
                                                ALU.mult, ALU.add)
                        rec2 = p1.tile([128, 1], F32, name=f"it{it}_rc_{qti}", tag="ln_rc")
                        nc.vector.reciprocal(rec2[:], dd[:])
                        o_t = p2.tile([128, D], F32, name=f"it{it}_o_{qti}", tag="o")
                        nc.vector.tensor_scalar(o_t[:], y_t[:], mean[:], rec2[:],
                                                ALU.subtract, ALU.mult)
                        nc.vector.tensor_mul(o_t[:], o_t[:], lna_t[:])
                        nc.vector.tensor_add(o_t[:], o_t[:], lnb_t[:])
                        nc.sync.dma_start(out_d[qti * 128:(qti + 1) * 128, :], o_t[:])

    nc.compile()
    return nc


def _get_nc():
    if "nc" not in _CACHE:
        _CACHE["nc"] = build()
    return _CACHE["nc"]


def kernel(q, w_qs, w_ks, w_vs, proj_w, proj_b, ln_a, ln_b, **kw):
    q = np.asarray(q, dtype=np.float32)
    w_qs = np.asarray(w_qs, dtype=np.float32)
    w_ks = np.asarray(w_ks, dtype=np.float32)
    w_vs = np.asarray(w_vs, dtype=np.float32)
    proj_w = np.asarray(proj_w, dtype=np.float32)
    proj_b = np.asarray(proj_b, dtype=np.float32)
    ln_a = np.asarray(ln_a, dtype=np.float32)
    ln_b = np.asarray(ln_b, dtype=np.float32)

    wq_all = np.ascontiguousarray(w_qs.transpose(1, 0, 2).reshape(D, H * DK))
    wk_all = np.ascontiguousarray(w_ks.transpose(1, 0, 2).reshape(D, H * DK))
    wv_aug = np.zeros((D, H, 65), dtype=np.float32)
    wv_aug[:, :, :64] = w_vs.transpose(1, 0, 2)
    wv_aug = np.ascontiguousarray(wv_aug.reshape(D, H * 65))
    pwT = np.ascontiguousarray(proj_w.T)
    ones64 = np.ones((1, 64), dtype=np.float32)
    lna = np.ascontiguousarray(ln_a[None, :])
    lnb = np.ascontiguousarray(ln_b[None, :])

    in_maps = []
    for c in range(8):
        b, half = c // 2, c % 2
        qbT = q[b].T  # [D, L]
        qt_c = np.ascontiguousarray(
            np.concatenate(
                [qbT[:, half * HALF:(half + 1) * HALF],
                 qbT[:, (1 - half) * HALF:(2 - half) * HALF]],
                axis=1,
            )
        )
        qres_c = np.ascontiguousarray(
            q[b, half * HALF:(half + 1) * HALF, :] + proj_b[None, :]
        )
        in_maps.append({
            "qt": qt_c, "qres": qres_c,
            "wq": wq_all, "wk": wk_all, "wv": wv_aug, "pw": pwT,
            "lna": lna, "lnb": lnb, "ones64": ones64,
        })

    nc = _get_nc()
    res = run_bass_kernel_spmd(nc, in_maps, core_ids=list(range(8))).results

    out = np.empty((B, L, D), dtype=np.float32)
    for c in range(8):
        b, half = c // 2, c % 2
        out[b, half * HALF:(half + 1) * HALF, :] = res[c]["out"]
    return out



# revision 2
# speedup vs baseline: 1.0318x; 1.0318x over previous
"""MultiHeadAttention TRN2 Bass kernel (8 NeuronCores) — fp8 DoubleRow version.

Sharding: core c = (batch b = c//2, query-half = c%2). Each core computes
K/V for its full batch (2048 keys) and attention + output projection + LN
for its 1024 query rows. No collectives; host gathers per-core outputs.

Device math (attention path in fp8e4m3 with DoubleRow matmuls; residual and
LayerNorm in fp32):
  scaling: wq,wk x8; wv x16; proj_w x8; qres x128; LN eps x128 (LN output is
  invariant under common scaling of y and eps).
  QhT/KhT in dk-split layout [32 part, ko=2, m] per head so the dk=64-deep
  score matmul is one DoubleRow instr (Ki=32, Ko=2, base partition 32h).
  Vaug [m, 65]: col 64 = ones -> the AV DoubleRow over m-pairs accumulates
  numerator rows (0..63) and the softmax denominator (row 64) together.
  exp: split across ACT (native Exp -> fp8), DVE and GPSIMD (Schraudolph:
  uint8 = round(s*C1 + C2) is the fp8e4m3 encoding of ~exp(s*g); GPSIMD
  cannot read PSUM so its chunks are DMA-staged to SBUF first).
  denom: copy row 64 -> K=1 ones-matmul broadcast -> reciprocal_approx ->
  multiply (head-parity 1 lands in ct partitions 64..127 via SBUF DMA).
"""
import numpy as np
import ml_dtypes

import concourse.bass as bass
import concourse.mybir as mybir
import concourse.tile as tile
from concourse import bacc
from concourse.bass_utils import run_bass_kernel_spmd

F32 = mybir.dt.float32
F32R = mybir.dt.float32r
F8 = mybir.dt.float8e4
U8 = mybir.dt.uint8
AF = mybir.ActivationFunctionType
ALU = mybir.AluOpType
DRM = mybir.MatmulPerfMode.DoubleRow

B, L, D = 4, 2048, 1024
H, DK = 16, 64
HALF = 1024
TEMPER = 32.0
PHASES = 4
HP = 4                # heads per phase
MT = 16               # m-tiles of 128
LN_EPS = 1e-3

S_QK = 8.0
S_V = 16.0
S_P = 8.0
SCL = S_V * S_P       # = 128; qres and eps pre-scaled by this
G_EXP = 1.0 / (TEMPER * S_QK * S_QK)
C1 = 8.0 * G_EXP / np.log(2.0)
C2 = 56.0 - 0.4327

_CACHE = {}


class Bal:
    """Static engine load balancer over ACT / DVE / GPSIMD."""

    def __init__(self, nc):
        self.nc = nc
        self.t = {"act": 0.0, "dve": 0.0, "pool": 0.0}
        self.eng = {"act": nc.scalar, "dve": nc.vector, "pool": nc.gpsimd}

    def pick(self, costs):
        k = min(costs, key=lambda e: self.t[e] + costs[e])
        self.t[k] += costs[k]
        return k

    def add(self, e, c):
        self.t[e] += c

    def copy(self, out_ap, in_ap, free):
        """psum/sbuf -> sbuf copy with dtype convert (ACT or DVE)."""
        e = self.pick({"act": free * 0.833 + 180, "dve": free * 1.042 + 300})
        if e == "act":
            self.nc.scalar.copy(out_ap, in_ap)
        else:
            self.nc.vector.tensor_copy(out_ap, in_ap)


def build(iters=1):
    nc = bacc.Bacc(None, target_bir_lowering=False)
    qt_d = nc.dram_tensor("qt", [D, L], F8, kind="ExternalInput")
    qres_d = nc.dram_tensor("qres", [HALF, D], F32, kind="ExternalInput")
    wq_d = nc.dram_tensor("wq", [D, H * DK], F8, kind="ExternalInput")
    wk_d = nc.dram_tensor("wk", [D, H * DK], F8, kind="ExternalInput")
    wv_d = nc.dram_tensor("wv", [D, PHASES * HP * 80], F8, kind="ExternalInput")
    pw_d = nc.dram_tensor("pw", [D, D], F8, kind="ExternalInput")
    lna_d = nc.dram_tensor("lna", [1, D], F32, kind="ExternalInput")
    lnb_d = nc.dram_tensor("lnb", [1, D], F32, kind="ExternalInput")
    out_d = nc.dram_tensor("out", [HALF, D], F32, kind="ExternalOutput")
    scr_d = nc.dram_tensor("scr", [H * 2, 512], F32, kind="Internal")

    with tile.TileContext(nc) as tc:
        with (
            tc.tile_pool(name="p1", bufs=1) as p1,
            tc.tile_pool(name="p2", bufs=2) as p2,
            tc.tile_pool(name="pq", bufs=8) as pq,
            tc.tile_pool(name="pex", bufs=4) as pex,
            tc.tile_pool(name="psS", bufs=2, space="PSUM") as psS,
            tc.tile_pool(name="psO", bufs=2, space="PSUM") as psO,
            tc.tile_pool(name="psP", bufs=2, space="PSUM") as psP,
        ):
            bal = Bal(nc)

            # ---- constants / persistent tiles ----
            ones8f = p1.tile([128, MT], F32, name="ones8f")
            nc.vector.memset(ones8f[:], 1.0)
            ones8 = p1.tile([128, MT], F8, name="ones8")
            nc.vector.tensor_copy(ones8[:], ones8f[:])
            lna_t = p1.tile([128, D], F32, name="lna_t")
            nc.sync.dma_start(lna_t[:], lna_d[:].to_broadcast([128, D]))
            lnb_t = p1.tile([128, D], F32, name="lnb_t")
            nc.sync.dma_start(lnb_t[:], lnb_d[:].to_broadcast([128, D]))
            qt_t = p1.tile([128, 8, L], F8, name="qt_t")
            nc.sync.dma_start(
                qt_t[:], qt_d[:].rearrange("(dj pp) m -> pp dj m", pp=128)
            )
            ct_t = p1.tile([128, H // 2, HALF], F8, name="ct_t")

            for it in range(iters):
                nm = f"it{it}"
                tiles = {}

                def load_phase(p):
                    wq_t = p2.tile([128, 8, HP * DK], F8,
                                   name=f"{nm}_wq{p}", tag="wq")
                    nc.sync.dma_start(
                        wq_t[:],
                        wq_d[:, p * 256:(p + 1) * 256].rearrange(
                            "(dj pp) f -> pp dj f", pp=128),
                    )
                    wk_t = p2.tile([128, 8, HP * DK], F8,
                                   name=f"{nm}_wk{p}", tag="wk")
                    nc.sync.dma_start(
                        wk_t[:],
                        wk_d[:, p * 256:(p + 1) * 256].rearrange(
                            "(dj pp) f -> pp dj f", pp=128),
                    )
                    wv_t = p2.tile([128, 8, HP * 80], F8,
                                   name=f"{nm}_wv{p}", tag="wv")
                    nc.sync.dma_start(
                        wv_t[:],
                        wv_d[:, p * 320:(p + 1) * 320].rearrange(
                            "(dj pp) f -> pp dj f", pp=128),
                    )
                    # kht/qht: [32 part, head, ko, m]; psum eviction writes
                    # [:, :, hf, mslice] (the projection psum partitions are
                    # (h*32+j) so partition p maps to head p//32, row p%32)
                    kht = p2.tile([128, 2, L], F8, name=f"{nm}_kht{p}",
                                  tag="kht")
                    qht = p2.tile([128, 2, HALF], F8, name=f"{nm}_qht{p}",
                                  tag="qht")
                    vaug = p2.tile([128, MT, HP, 80], F8, name=f"{nm}_va{p}",
                                   tag="vaug")
                    for hh in range(HP):
                        nc.vector.tensor_copy(vaug[:, :, hh, 64], ones8[:])
                    tiles[p] = (wq_t, wk_t, wv_t, kht, qht, vaug)

                def proj_groups(p):
                    """Closures, one per projection psum group."""
                    wq_t, wk_t, wv_t, kht, qht, vaug = tiles[p]

                    def kq_group(kind, mc, hf):
                        def go():
                            w_t = wk_t if kind == "k" else wq_t
                            dst = kht if kind == "k" else qht
                            ps = psP.tile(
                                [128, 512], F32,
                                name=f"{nm}_pp{p}_{kind}{mc}_{hf}", tag="pp")
                            for j in range(4):
                                nc.tensor.matmul(
                                    ps[:],
                                    w_t[:, 2 * j:2 * j + 2,
                                        128 * hf:128 * hf + 128],
                                    qt_t[:, 2 * j:2 * j + 2,
                                         mc * 512:(mc + 1) * 512],
                                    start=(j == 0), stop=(j == 3),
                                    perf_mode=DRM,
                                )
                            bal.copy(dst[:, hf, mc * 512:(mc + 1) * 512],
                                     ps[:], 512)
                        return go

                    def v_group(ms):
                        def go():
                            ps = psP.tile([128, HP, 80], F32,
                                          name=f"{nm}_vp{p}_{ms}", tag="pp")
                            for j in range(4):
                                nc.tensor.matmul(
                                    ps[:],
                                    qt_t[:, 2 * j:2 * j + 2,
                                         ms * 128:(ms + 1) * 128],
                                    wv_t[:, 2 * j:2 * j + 2, :],
                                    start=(j == 0), stop=(j == 3),
                                    perf_mode=DRM,
                                )
                            bal.copy(vaug[:, ms, :, 0:64], ps[:, :, 0:64], 256)
                        return go

                    for mc in range(4):
                        for hf in range(2):
                            yield kq_group("k", mc, hf)
                    for mc in range(2):
                        for hf in range(2):
                            yield kq_group("q", mc, hf)
                    for ms in range(MT):
                        yield v_group(ms)

                pending = []

                def attention(p, feeder):
                    _, _, _, kht, qht, vaug = tiles[p]
                    for hh in range(HP):
                        g = HP * p + hh
                        cj, par = g // 2, g % 2
                        for qc in range(2):
                            ot = psO.tile([65, 512], F32,
                                          name=f"{nm}_ot{g}_{qc}", tag="ot")
                            for i in range(8):  # m-pair chunks
                                if i in (2, 6) and pending:
                                    pending.pop(0)()
                                if i in (1, 3, 5, 7):
                                    nxt = next(feeder, None)
                                    if nxt is not None:
                                        nxt()
                                sp = psS.tile([128, 2, 512], F32,
                                              name=f"{nm}_sp{g}_{qc}_{i}",
                                              tag="sc")
                                for u in range(2):
                                    c = 2 * i + u
                                    nc.tensor.matmul(
                                        sp[:, u, :],
                                        kht[32 * hh:32 * hh + 32, :,
                                            c * 128:(c + 1) * 128],
                                        qht[32 * hh:32 * hh + 32, :,
                                            qc * 512:(qc + 1) * 512],
                                        start=True, stop=True, perf_mode=DRM,
                                        tile_position=(32 * hh, 0),
                                    )
                                ex = pex.tile([128, 2, 512], U8,
                                              name=f"{nm}_ex{g}_{qc}_{i}",
                                              tag="ex")
                                e = bal.pick({"act": 1030, "dve": 1370})
                                if e == "act":
                                    nc.scalar.activation(
                                        ex.bitcast(F8)[:, :, :], sp[:],
                                        AF.Exp, scale=G_EXP)
                                else:
                                    nc.vector.tensor_scalar(
                                        ex[:], sp[:], C1, C2,
                                        ALU.mult, ALU.add)
                                nc.tensor.matmul(
                                    ot[:],
                                    vaug[:, 2 * i:2 * i + 2, hh, 0:65],
                                    ex.bitcast(F8)[:, :, :],
                                    start=(i == 0), stop=(i == 7),
                                    perf_mode=DRM,
                                )
                            # ---- denominator chain: reciprocal of the
                            # psum denominator row, DRAM-roundtrip
                            # partition-broadcast, then normalize ----
                            rr = p2.tile([1, 512], F32,
                                         name=f"{nm}_rr{g}_{qc}", tag="den")
                            nc.vector.reciprocal(rr[:], ot[64:65, :])
                            bal.add("dve", 790)
                            srow = 2 * g + qc
                            nc.sync.dma_start(scr_d[srow:srow + 1, :], rr[:])

                            def s2(srow=srow, g=g, qc=qc, state={}):
                                rb = p2.tile([64, 512], F32,
                                             name=f"{nm}_rb{g}_{qc}",
                                             tag="rec")
                                nc.sync.dma_start(
                                    rb[:],
                                    scr_d[srow:srow + 1, :].to_broadcast(
                                        [64, 512]))
                                s2.rb = rb

                            def s3(ot=ot, s2=s2, cj=cj, par=par, qc=qc, g=g):
                                qs = slice(qc * 512, (qc + 1) * 512)
                                bal.add("dve", 790)
                                if par == 0:
                                    nc.vector.tensor_tensor(
                                        ct_t[0:64, cj, qs], ot[0:64, :],
                                        s2.rb[:], ALU.mult)
                                else:
                                    stg = p2.tile([64, 512], F8,
                                                  name=f"{nm}_st{g}_{qc}",
                                                  tag="stg")
                                    nc.vector.tensor_tensor(
                                        stg[:], ot[0:64, :], s2.rb[:],
                                        ALU.mult)
                                    nc.sync.dma_start(
                                        ct_t[64:128, cj, qs], stg[:])

                            pending.extend([s2, s3])
                    while pending:
                        pending.pop(0)()

                # ---- run phases; stage B of p+1 feeds into stage C of p ----
                load_phase(0)
                for g_ in proj_groups(0):
                    g_()
                pw_ts = []
                qres_ts = []
                for p in range(PHASES):
                    if p + 1 < PHASES:
                        load_phase(p + 1)
                        feeder = proj_groups(p + 1)
                    else:
                        feeder = iter(())
                        for oc in range(2):
                            pw_t = p2.tile([128, 8, 512], F8,
                                           name=f"{nm}_pw{oc}", tag="pw")
                            nc.sync.dma_start(
                                pw_t[:],
                                pw_d[:, oc * 512:(oc + 1) * 512].rearrange(
                                    "(dj pp) f -> pp dj f", pp=128),
                            )
                            pw_ts.append(pw_t)
                        for qti in range(8):
                            qr = pq.tile([128, D], F32,
                                         name=f"{nm}_qr{qti}", tag="qr")
                            nc.sync.dma_start(
                                qr[:], qres_d[qti * 128:(qti + 1) * 128, :])
                            qres_ts.append(qr)
                    attention(p, feeder)

                # ---- output projection + residual + LayerNorm ----
                for qti in range(8):
                    yts = p2.tile([128, D], F32, name=f"{nm}_y{qti}", tag="y")
                    sums = p1.tile([128, 2], F32, name=f"{nm}_sm{qti}",
                                   tag=f"sm{qti}")
                    for oc in range(2):
                        ps = psP.tile([128, 512], F32,
                                      name=f"{nm}_yp{qti}_{oc}", tag="pp")
                        for j in range(4):
                            nc.tensor.matmul(
                                ps[:],
                                ct_t[:, 2 * j:2 * j + 2,
                                     qti * 128:(qti + 1) * 128],
                                pw_ts[oc][:, 2 * j:2 * j + 2, :],
                                start=(j == 0), stop=(j == 3), perf_mode=DRM,
                            )
                        qslc = qres_ts[qti][:, oc * 512:(oc + 1) * 512]
                        bal.add("dve", 790)
                        nc.vector.scalar_tensor_tensor(
                            yts[:, oc * 512:(oc + 1) * 512], ps[:], 1.0,
                            qslc, ALU.mult, ALU.add,
                            accum_out=sums[:, oc:oc + 1])
                    # LayerNorm (y and eps are SCL-scaled; output invariant)
                    negmean = p1.tile([128, 1], F32, name=f"{nm}_nm{qti}",
                                      tag=f"nm{qti}")
                    nc.vector.tensor_scalar(negmean[:], sums[:, 0:1],
                                            sums[:, 1:2], -1.0 / D,
                                            ALU.add, ALU.mult)
                    bal.add("dve", 150)
                    ss = p1.tile([128, 2], F32, name=f"{nm}_ss{qti}",
                                 tag=f"ss{qti}")
                    for oc in range(2):
                        sq = psS.tile([128, 2, 512], F32,
                                      name=f"{nm}_sq{qti}_{oc}", tag="sc")
                        nc.scalar.activation(
                            sq[:, 0, :], yts[:, oc * 512:(oc + 1) * 512],
                            AF.Square, bias=negmean[:],
                            accum_out=ss[:, oc:oc + 1])
                        bal.add("act", 800)
                    sst = p1.tile([128, 1], F32, name=f"{nm}_sst{qti}",
                                  tag=f"sst{qti}")
                    nc.vector.tensor_tensor(sst[:], ss[:, 0:1], ss[:, 1:2],
                                            ALU.add)
                    sigma = p1.tile([128, 1], F32, name=f"{nm}_sg{qti}",
                                    tag=f"sg{qti}")
                    nc.scalar.activation(sigma[:], sst[:], AF.Sqrt,
                                         scale=1.0 / (D - 1))
                    var = p1.tile([128, 1], F32, name=f"{nm}_var{qti}",
                                  tag=f"var{qti}")
                    nc.vector.tensor_scalar_mul(var[:], sst[:], 1.0 / (D - 1))
                    rs = p1.tile([128, 1], F32, name=f"{nm}_rs{qti}",
                                 tag=f"rs{qti}")
                    nc.vector.reciprocal(rs[:], sigma[:])
                    t1 = p1.tile([128, 1], F32, name=f"{nm}_t1{qti}",
                                 tag=f"t1{qti}")
                    nc.vector.scalar_tensor_tensor(t1[:], var[:], rs[:],
                                                   sigma[:], ALU.mult,
                                                   ALU.add)
                    dd = p1.tile([128, 1], F32, name=f"{nm}_dd{qti}",
                                 tag=f"dd{qti}")
                    nc.vector.tensor_scalar(dd[:], t1[:], 0.5, LN_EPS * SCL,
                                            ALU.mult, ALU.add)
                    rec2 = p1.tile([128, 1], F32, name=f"{nm}_r2{qti}",
                                   tag=f"r2{qti}")
                    nc.vector.reciprocal(rec2[:], dd[:])
                    nb = p1.tile([128, 1], F32, name=f"{nm}_nb{qti}",
                                 tag=f"nb{qti}")
                    nc.vector.tensor_tensor(nb[:], negmean[:], rec2[:],
                                            ALU.mult)
                    bal.add("dve", 900)
                    o1 = p2.tile([128, D], F32, name=f"{nm}_o1{qti}",
                                 tag="o1")
                    e = bal.pick({"act": 1040, "dve": 1370, "pool": 1520})
                    if e == "act":
                        nc.scalar.activation(o1[:], yts[:], AF.Identity,
                                             bias=nb[:], scale=rec2[:])
                    else:
                        bal.eng[e].tensor_scalar(o1[:], yts[:], rec2[:],
                                                 nb[:], ALU.mult, ALU.add)
                    e = bal.pick({"dve": 1370, "pool": 1520})
                    bal.eng[e].tensor_tensor(o1[:], o1[:], lna_t[:], ALU.mult)
                    e = bal.pick({"dve": 1370, "pool": 1520})
                    bal.eng[e].tensor_tensor(o1[:], o1[:], lnb_t[:], ALU.add)
                    nc.sync.dma_start(out_d[qti * 128:(qti + 1) * 128, :],
                                      o1[:])

    nc.compile()
    build.last_bal = bal.t
    return nc


def _get_nc():
    if "nc" not in _CACHE:
        _CACHE["nc"] = build()
    return _CACHE["nc"]


def prep_inputs(q, w_qs, w_ks, w_vs, proj_w, proj_b, ln_a, ln_b):
    f8 = ml_dtypes.float8_e4m3
    q = np.asarray(q, dtype=np.float32)
    w_qs = np.asarray(w_qs, dtype=np.float32)
    w_ks = np.asarray(w_ks, dtype=np.float32)
    w_vs = np.asarray(w_vs, dtype=np.float32)
    proj_w = np.asarray(proj_w, dtype=np.float32)
    proj_b = np.asarray(proj_b, dtype=np.float32)
    ln_a = np.asarray(ln_a, dtype=np.float32)
    ln_b = np.asarray(ln_b, dtype=np.float32)

    # dk-split column permutation for wq/wk: per phase 256 cols =
    # [4 heads x dk 0:32 | 4 heads x dk 32:64]
    def perm_w(w):
        wt = (w * S_QK).transpose(1, 0, 2)  # [D, H, DK]
        arr = np.empty((D, PHASES, 2, HP, 32), np.float32)
        for p in range(PHASES):
            for hh in range(HP):
                g = HP * p + hh
                arr[:, p, 0, hh, :] = wt[:, g, 0:32]
                arr[:, p, 1, hh, :] = wt[:, g, 32:64]
        return np.ascontiguousarray(arr.reshape(D, H * DK)).astype(f8)

    wq_host = perm_w(w_qs)
    wk_host = perm_w(w_ks)

    wvt = (w_vs * S_V).transpose(1, 0, 2)  # [D, H, 64]
    wv_arr = np.zeros((D, PHASES, HP, 80), np.float32)
    for p in range(PHASES):
        for hh in range(HP):
            wv_arr[:, p, hh, 0:64] = wvt[:, HP * p + hh, :]
    wv_host = np.ascontiguousarray(wv_arr.reshape(D, PHASES * HP * 80)) \
        .astype(f8)

    pw_host = np.ascontiguousarray(proj_w.T * S_P).astype(f8)
    lna = np.ascontiguousarray(ln_a[None, :])
    lnb = np.ascontiguousarray(ln_b[None, :])

    in_maps = []
    for c in range(8):
        b, half = c // 2, c % 2
        qbT = q[b].T  # [D, L]
        qt_c = np.ascontiguousarray(
            np.concatenate(
                [qbT[:, half * HALF:(half + 1) * HALF],
                 qbT[:, (1 - half) * HALF:(2 - half) * HALF]],
                axis=1,
            )
        ).astype(f8)
        qres_c = np.ascontiguousarray(
            (q[b, half * HALF:(half + 1) * HALF, :] + proj_b[None, :]) * SCL
        )
        in_maps.append({
            "qt": qt_c, "qres": qres_c,
            "wq": wq_host, "wk": wk_host, "wv": wv_host, "pw": pw_host,
            "lna": lna, "lnb": lnb,
        })
    return in_maps


def kernel(q, w_qs, w_ks, w_vs, proj_w, proj_b, ln_a, ln_b, **kw):
    in_maps = prep_inputs(q, w_qs, w_ks, w_vs, proj_w, proj_b, ln_a, ln_b)
    nc = _get_nc()
    res = run_bass_kernel_spmd(nc, in_maps, core_ids=list(range(8))).results

    out = np.empty((B, L, D), dtype=np.float32)
    for c in range(8):
        b, half = c // 2, c % 2
        out[b, half * HALF:(half + 1) * HALF, :] = res[c]["out"]
    return out


# revision 4
# speedup vs baseline: 1.1201x; 1.0855x over previous
"""MultiHeadAttention TRN2 Bass kernel (8 NeuronCores) — fp8 DoubleRow version.

Sharding: core c = (batch b = c//2, query-half = c%2). Each core computes
K/V for its full batch (2048 keys) and attention + output projection + LN
for its 1024 query rows. No collectives; host gathers per-core outputs.

Device math (attention path in fp8e4m3 with DoubleRow matmuls; residual and
LayerNorm in fp32):
  scaling: wq,wk x8; wv x16; proj_w x8; qres x128; LN eps x128 (LN output is
  invariant under common scaling of y and eps).
  QhT/KhT in dk-split layout [32 part, ko=2, m] per head so the dk=64-deep
  score matmul is one DoubleRow instr (Ki=32, Ko=2, base partition 32h).
  Vaug [m, 65]: col 64 = ones -> the AV DoubleRow over m-pairs accumulates
  numerator rows (0..63) and the softmax denominator (row 64) together.
  exp: split across ACT (native Exp -> fp8), DVE and GPSIMD (Schraudolph:
  uint8 = round(s*C1 + C2) is the fp8e4m3 encoding of ~exp(s*g); GPSIMD
  cannot read PSUM so its chunks are DMA-staged to SBUF first).
  denom: copy row 64 -> K=1 ones-matmul broadcast -> reciprocal_approx ->
  multiply (head-parity 1 lands in ct partitions 64..127 via SBUF DMA).
"""
import numpy as np
import ml_dtypes

import concourse.bass as bass
import concourse.mybir as mybir
import concourse.tile as tile
from concourse import bacc
from concourse.bass_utils import run_bass_kernel_spmd

F32 = mybir.dt.float32
F32R = mybir.dt.float32r
F8 = mybir.dt.float8e4
U8 = mybir.dt.uint8
AF = mybir.ActivationFunctionType
ALU = mybir.AluOpType
DRM = mybir.MatmulPerfMode.DoubleRow

B, L, D = 4, 2048, 1024
H, DK = 16, 64
HALF = 1024
TEMPER = 32.0
PHASES = 4
HP = 4                # heads per phase
MT = 16               # m-tiles of 128
LN_EPS = 1e-3

S_QK = 8.0
S_V = 16.0
S_P = 8.0
SCL = S_V * S_P       # = 128; qres and eps pre-scaled by this
G_EXP = 1.0 / (TEMPER * S_QK * S_QK)
C1 = 8.0 * G_EXP / np.log(2.0)
C2 = 56.0 - 0.4327

_CACHE = {}


class Bal:
    """Static engine load balancer over ACT / DVE / GPSIMD."""

    def __init__(self, nc):
        self.nc = nc
        self.t = {"act": 0.0, "dve": 0.0, "pool": 0.0}
        # sim shows the estimator undercounts ACT relative to DVE
        self.skew = {"act": 1.10, "dve": 1.0, "pool": 1.0}
        self.eng = {"act": nc.scalar, "dve": nc.vector, "pool": nc.gpsimd}

    def pick(self, costs):
        k = min(costs, key=lambda e: (self.t[e] + costs[e]) * self.skew[e])
        self.t[k] += costs[k]
        return k

    def add(self, e, c):
        self.t[e] += c

    def copy(self, out_ap, in_ap, free):
        """psum/sbuf -> sbuf copy with dtype convert (ACT or DVE)."""
        e = self.pick({"act": free * 0.833 + 180, "dve": free * 1.042 + 300})
        if e == "act":
            self.nc.scalar.copy(out_ap, in_ap)
        else:
            self.nc.vector.tensor_copy(out_ap, in_ap)


def build(iters=1, ln_trivial=False):
    nc = bacc.Bacc(None, target_bir_lowering=False)
    # all inputs host-rearranged to partition-major contiguous layouts
    qt_d = nc.dram_tensor("qt", [128, 8, L], F8, kind="ExternalInput")
    qres_d = nc.dram_tensor("qres", [HALF, D], F32, kind="ExternalInput")
    wq_d = nc.dram_tensor("wq", [PHASES * 128, 8 * 256], F8,
                          kind="ExternalInput")
    wk_d = nc.dram_tensor("wk", [PHASES * 128, 8 * 256], F8,
                          kind="ExternalInput")
    wv_d = nc.dram_tensor("wv", [PHASES * 128, 8 * 320], F8,
                          kind="ExternalInput")
    pw_d = nc.dram_tensor("pw", [2 * 128, 8 * 512], F8, kind="ExternalInput")
    lna_d = nc.dram_tensor("lna", [1, D], F32, kind="ExternalInput")
    lnb_d = nc.dram_tensor("lnb", [1, D], F32, kind="ExternalInput")
    out_d = nc.dram_tensor("out", [HALF, D], F32, kind="ExternalOutput")
    scr_d = nc.dram_tensor("scr", [H * 2, 512], F32, kind="Internal")

    with tile.TileContext(nc) as tc:
        with (
            tc.tile_pool(name="p1", bufs=1) as p1,
            tc.tile_pool(name="p2", bufs=2) as p2,
            tc.tile_pool(name="pq", bufs=8) as pq,
            tc.tile_pool(name="pex", bufs=4) as pex,
            tc.tile_pool(name="psS", bufs=4, space="PSUM") as psS,
            tc.tile_pool(name="psO", bufs=2, space="PSUM") as psO,
            tc.tile_pool(name="psP", bufs=2, space="PSUM") as psP,
        ):
            bal = Bal(nc)

            # ---- constants / persistent tiles ----
            # DMA order matters: phase-0 K/Q projections need qt chunk 0 and
            # wq/wk first; lna/lnb are only needed at the very end.
            qt_t = p1.tile([128, 8, L], F8, name="qt_t")

            def load_qt(mc):
                nc.sync.dma_start(qt_t[:, :, mc * 512:(mc + 1) * 512],
                                  qt_d[:, :, mc * 512:(mc + 1) * 512])
            load_qt(0)
            ones8f = p1.tile([128, MT], F32, name="ones8f")
            nc.vector.memset(ones8f[:], 1.0)
            ones8 = p1.tile([128, MT], F8, name="ones8")
            nc.vector.tensor_copy(ones8[:], ones8f[:])
            ct_t = p1.tile([128, H // 2, HALF], F8, name="ct_t")

            for it in range(iters):
                nm = f"it{it}"
                tiles = {}

                def load_phase(p):
                    wq_t = p2.tile([128, 8, HP * DK], F8,
                                   name=f"{nm}_wq{p}", tag="wq")
                    nc.sync.dma_start(
                        wq_t[:],
                        wq_d[p * 128:(p + 1) * 128, :].rearrange(
                            "pp (dj f) -> pp dj f", dj=8))
                    wk_t = p2.tile([128, 8, HP * DK], F8,
                                   name=f"{nm}_wk{p}", tag="wk")
                    nc.sync.dma_start(
                        wk_t[:],
                        wk_d[p * 128:(p + 1) * 128, :].rearrange(
                            "pp (dj f) -> pp dj f", dj=8))
                    wv_t = p2.tile([128, 8, HP * 80], F8,
                                   name=f"{nm}_wv{p}", tag="wv")
                    nc.sync.dma_start(
                        wv_t[:],
                        wv_d[p * 128:(p + 1) * 128, :].rearrange(
                            "pp (dj f) -> pp dj f", dj=8))
                    # kht/qht: [32 part, head, ko, m]; psum eviction writes
                    # [:, :, hf, mslice] (the projection psum partitions are
                    # (h*32+j) so partition p maps to head p//32, row p%32)
                    kht = p2.tile([128, 2, L], F8, name=f"{nm}_kht{p}",
                                  tag="kht")
                    qht = p2.tile([128, 2, HALF], F8, name=f"{nm}_qht{p}",
                                  tag="qht")
                    vaug = p2.tile([128, MT, HP, 80], F8, name=f"{nm}_va{p}",
                                   tag="vaug")
                    for hh in range(HP):
                        nc.vector.tensor_copy(vaug[:, :, hh, 64], ones8[:])
                    tiles[p] = (wq_t, wk_t, wv_t, kht, qht, vaug)

                def proj_groups(p):
                    """Closures, one per projection psum group."""
                    wq_t, wk_t, wv_t, kht, qht, vaug = tiles[p]

                    def kq_group(kind, mc, hf):
                        def go():
                            w_t = wk_t if kind == "k" else wq_t
                            dst = kht if kind == "k" else qht
                            ps = psP.tile(
                                [128, 512], F32,
                                name=f"{nm}_pp{p}_{kind}{mc}_{hf}", tag="pp")
                            for j in range(4):
                                nc.tensor.matmul(
                                    ps[:],
                                    w_t[:, 2 * j:2 * j + 2,
                                        128 * hf:128 * hf + 128],
                                    qt_t[:, 2 * j:2 * j + 2,
                                         mc * 512:(mc + 1) * 512],
                                    start=(j == 0), stop=(j == 3),
                                    perf_mode=DRM,
                                )
                            bal.copy(dst[:, hf, mc * 512:(mc + 1) * 512],
                                     ps[:], 512)
                        return go

                    def v_group(ms):
                        def go():
                            ps = psP.tile([128, HP, 80], F32,
                                          name=f"{nm}_vp{p}_{ms}", tag="pp")
                            for j in range(4):
                                nc.tensor.matmul(
                                    ps[:],
                                    qt_t[:, 2 * j:2 * j + 2,
                                         ms * 128:(ms + 1) * 128],
                                    wv_t[:, 2 * j:2 * j + 2, :],
                                    start=(j == 0), stop=(j == 3),
                                    perf_mode=DRM,
                                )
                            bal.copy(vaug[:, ms, :, 0:64], ps[:, :, 0:64], 256)
                        return go

                    for mc in range(4):
                        for hf in range(2):
                            yield kq_group("k", mc, hf)
                    for mc in range(2):
                        for hf in range(2):
                            yield kq_group("q", mc, hf)
                    for ms in range(MT):
                        yield v_group(ms)

                pending = []

                def attention(p, feeder, late_feeder=None):
                    _, _, _, kht, qht, vaug = tiles[p]
                    if p + 1 < PHASES:
                        order = [(hh, qc) for hh in range(HP)
                                 for qc in range(2)]
                    else:
                        # qc-major so all q-half-0 ct columns finish early
                        # and the output projection can overlap attention
                        order = [(hh, 0) for hh in range(HP)] + \
                                [(hh, 1) for hh in range(HP)]
                    for oidx, (hh, qc) in enumerate(order):
                        if True:
                            g = HP * p + hh
                            cj, par = g // 2, g % 2
                            ot = psO.tile([65, 512], F32,
                                          name=f"{nm}_ot{g}_{qc}", tag="ot")
                            for i in range(8):  # m-pair chunks
                                if i == 4 and pending:
                                    pending.pop(0)()
                                if i != 4:
                                    late = (late_feeder is not None
                                            and oidx >= 5)
                                    if late or i in (1, 3, 5, 7):
                                        src = late_feeder if late else feeder
                                        nxt = next(src, None)
                                        if nxt is not None:
                                            nxt()
                                ex = pex.tile([128, 2, 512], U8,
                                              name=f"{nm}_ex{g}_{qc}_{i}",
                                              tag="ex")
                                for u in range(2):
                                    c = 2 * i + u
                                    sp = psS.tile(
                                        [128, 512], F32,
                                        name=f"{nm}_sp{g}_{qc}_{c}",
                                        tag="sc")
                                    nc.tensor.matmul(
                                        sp[:],
                                        kht[32 * hh:32 * hh + 32, :,
                                            c * 128:(c + 1) * 128],
                                        qht[32 * hh:32 * hh + 32, :,
                                            qc * 512:(qc + 1) * 512],
                                        start=True, stop=True, perf_mode=DRM,
                                        tile_position=(32 * hh, 0),
                                    )
                                    e = bal.pick({"act": 613, "dve": 833})
                                    if e == "act":
                                        nc.scalar.activation(
                                            ex.bitcast(F8)[:, u, :], sp[:],
                                            AF.Exp, scale=G_EXP)
                                    else:
                                        nc.vector.tensor_scalar(
                                            ex[:, u, :], sp[:], C1, C2,
                                            ALU.mult, ALU.add)
                                nc.tensor.matmul(
                                    ot[:],
                                    vaug[:, 2 * i:2 * i + 2, hh, 0:65],
                                    ex.bitcast(F8)[:, :, :],
                                    start=(i == 0), stop=(i == 7),
                                    perf_mode=DRM,
                                )
                            # ---- denominator chain: reciprocal of the
                            # psum denominator row, DRAM-roundtrip
                            # partition-broadcast, then normalize ----
                            rr = p2.tile([1, 512], F32,
                                         name=f"{nm}_rr{g}_{qc}", tag="den")
                            nc.vector.reciprocal(rr[:], ot[64:65, :])
                            bal.add("dve", 790)
                            srow = 2 * g + qc
                            nc.sync.dma_start(scr_d[srow:srow + 1, :], rr[:])

                            rb = p2.tile([64, 512], F32,
                                         name=f"{nm}_rb{g}_{qc}", tag="rec")
                            nc.sync.dma_start(
                                rb[:],
                                scr_d[srow:srow + 1, :].to_broadcast(
                                    [64, 512]))
                            # evict the numerator rows to SBUF so the
                            # normalize multiply can run on idle GPSIMD
                            ots = p2.tile([64, 512], F32,
                                          name=f"{nm}_os{g}_{qc}", tag="ots")
                            bal.copy(ots[:], ot[0:64, :], 512)

                            def s3(ots=ots, rb=rb, cj=cj, par=par, qc=qc,
                                   g=g):
                                qs = slice(qc * 512, (qc + 1) * 512)
                                bal.add("pool", 810)
                                if par == 0:
                                    nc.gpsimd.tensor_tensor(
                                        ct_t[0:64, cj, qs], ots[:],
                                        rb[:], ALU.mult)
                                else:
                                    stg = p2.tile([64, 512], F8,
                                                  name=f"{nm}_st{g}_{qc}",
                                                  tag="stg")
                                    nc.gpsimd.tensor_tensor(
                                        stg[:], ots[:], rb[:],
                                        ALU.mult)
                                    nc.sync.dma_start(
                                        ct_t[64:128, cj, qs], stg[:])

                            pending.append(s3)
                    while pending:
                        pending.pop(0)()

                # ---- output projection + residual + LayerNorm ----
                # Per-tile pass closures; tiles 0-3 are emitted early,
                # interleaved with phase-3 second-half attention.
                ytss = {}
                nmb = p1.tile([128, 8], F32, name=f"{nm}_nmb")
                sstb = p1.tile([128, 8], F32, name=f"{nm}_sstb")
                rc2b = p1.tile([128, 8], F32, name=f"{nm}_rc2b")
                nbb = p1.tile([128, 8], F32, name=f"{nm}_nbb")

                def d_pass1(qti, jmax=4):
                    yts = p2.tile([128, D], F32, name=f"{nm}_y{qti}",
                                  tag=f"y{qti % 4}")
                    ytss[qti] = yts
                    sums = p1.tile([128, 2], F32, name=f"{nm}_sm{qti}",
                                   tag=f"sm{qti}")
                    for oc in range(2):
                        ps = psP.tile([128, 512], F32,
                                      name=f"{nm}_yp{qti}_{oc}", tag="pp")
                        for j in range(jmax):
                            nc.tensor.matmul(
                                ps[:],
                                ct_t[:, 2 * j:2 * j + 2,
                                     qti * 128:(qti + 1) * 128],
                                pw_ts[oc][:, 2 * j:2 * j + 2, :],
                                start=(j == 0), stop=(j == jmax - 1),
                                perf_mode=DRM,
                            )
                        qslc = qres_ts[qti][:, oc * 512:(oc + 1) * 512]
                        bal.add("dve", 790)
                        nc.vector.scalar_tensor_tensor(
                            yts[:, oc * 512:(oc + 1) * 512], ps[:], 1.0,
                            qslc, ALU.mult, ALU.add,
                            accum_out=(sums[:, oc:oc + 1] if jmax == 4
                                       else None))
                    if jmax == 4:
                        nc.vector.tensor_scalar(nmb[:, qti:qti + 1],
                                                sums[:, 0:1],
                                                sums[:, 1:2], -1.0 / D,
                                                ALU.add, ALU.mult)
                        bal.add("dve", 150)

                def d_pass1_fin(qti):
                    # last contraction pair (phase-3 heads) + final sums
                    yts = ytss[qti]
                    sums = p1.tile([128, 2], F32, name=f"{nm}_smf{qti}",
                                   tag=f"sm{qti}")
                    for oc in range(2):
                        ps = psP.tile([128, 512], F32,
                                      name=f"{nm}_ypf{qti}_{oc}", tag="pp")
                        nc.tensor.matmul(
                            ps[:],
                            ct_t[:, 6:8, qti * 128:(qti + 1) * 128],
                            pw_ts[oc][:, 6:8, :],
                            start=True, stop=True, perf_mode=DRM,
                        )
                        sl = slice(oc * 512, (oc + 1) * 512)
                        bal.add("dve", 790)
                        nc.vector.scalar_tensor_tensor(
                            yts[:, sl], ps[:], 1.0, yts[:, sl],
                            ALU.mult, ALU.add,
                            accum_out=sums[:, oc:oc + 1])
                    nc.vector.tensor_scalar(nmb[:, qti:qti + 1],
                                            sums[:, 0:1],
                                            sums[:, 1:2], -1.0 / D,
                                            ALU.add, ALU.mult)
                    bal.add("dve", 150)

                # variance accumulation (LayerNorm; y and eps SCL-scaled)
                def d_pass2(qti):
                    yts = ytss[qti]
                    negmean = nmb[:, qti:qti + 1]
                    ss = p1.tile([128, 2], F32, name=f"{nm}_ss{qti}",
                                 tag=f"ss{qti}")
                    if qti % 2 == 0:
                        # ACT path: sum (y - mu)^2 via Square with bias
                        for oc in range(2):
                            sq = psS.tile([128, 512], F32,
                                          name=f"{nm}_sq{qti}_{oc}", tag="sc")
                            nc.scalar.activation(
                                sq[:], yts[:, oc * 512:(oc + 1) * 512],
                                AF.Square, bias=negmean,
                                accum_out=ss[:, oc:oc + 1])
                            bal.add("act", 800)
                        nc.vector.tensor_tensor(sstb[:, qti:qti + 1],
                                                ss[:, 0:1],
                                                ss[:, 1:2], ALU.add)
                    else:
                        # DVE path: sum y^2, then subtract D*mu^2
                        dump = p2.tile([128, D], F32, name=f"{nm}_du{qti}",
                                       tag="du")
                        for oc in range(2):
                            sl = slice(oc * 512, (oc + 1) * 512)
                            nc.vector.scalar_tensor_tensor(
                                dump[:, sl], yts[:, sl], 1.0, yts[:, sl],
                                ALU.mult, ALU.mult,
                                accum_out=ss[:, oc:oc + 1])
                            bal.add("dve", 850)
                        musq = p1.tile([128, 1], F32, name=f"{nm}_mq{qti}",
                                       tag=f"mq{qti}")
                        nc.vector.tensor_tensor(musq[:], negmean,
                                                negmean, ALU.mult)
                        s01 = p1.tile([128, 1], F32, name=f"{nm}_s01{qti}",
                                      tag=f"s01{qti}")
                        nc.vector.tensor_tensor(s01[:], ss[:, 0:1],
                                                ss[:, 1:2], ALU.add)
                        nc.vector.scalar_tensor_tensor(
                            sstb[:, qti:qti + 1], musq[:], -float(D), s01[:],
                            ALU.mult, ALU.add)

                # batched sigma for all 8 tiles on DVE: rsqrt via the
                # int32 bit-trick + 2 Newton iterations (no ACT Sqrt, so
                # the ACT engine keeps the Exp table the whole kernel)
                def d_sigma_batch(base):
                    I32 = mybir.dt.int32
                    s = slice(base, base + 4)
                    varb = p1.tile([128, 8], F32, name=f"{nm}_varb")
                    nc.vector.tensor_scalar_mul(varb[:, s], sstb[:, s],
                                                1.0 / (D - 1))
                    sh = p1.tile([128, 8], I32, name=f"{nm}_sh")
                    nc.vector.tensor_scalar(sh[:, s], varb.bitcast(I32)[:, s],
                                            1, None, ALU.arith_shift_right)
                    nt = p1.tile([128, 8], I32, name=f"{nm}_nt")
                    nc.vector.tensor_scalar(nt[:, s], sh[:, s], 0xFFFFFFFF,
                                            None, ALU.bitwise_xor)
                    r0i = p1.tile([128, 8], I32, name=f"{nm}_r0i")
                    nc.vector.tensor_scalar(r0i[:, s], nt[:, s],
                                            0x5F3759DF + 1, None, ALU.add)
                    r = r0i.bitcast(F32)
                    for itn in range(2):
                        a = p1.tile([128, 8], F32, name=f"{nm}_nta{itn}")
                        nc.vector.tensor_tensor(a[:, s], r[:, s], r[:, s],
                                                ALU.mult)
                        nc.vector.tensor_tensor(a[:, s], a[:, s], varb[:, s],
                                                ALU.mult)
                        nc.vector.tensor_scalar(a[:, s], a[:, s], -0.5, 1.5,
                                                ALU.mult, ALU.add)
                        rn = p1.tile([128, 8], F32, name=f"{nm}_ntr{itn}")
                        nc.vector.tensor_tensor(rn[:, s], r[:, s], a[:, s],
                                                ALU.mult)
                        r = rn
                    sgb = p1.tile([128, 8], F32, name=f"{nm}_sgb")
                    nc.vector.tensor_tensor(sgb[:, s], varb[:, s], r[:, s],
                                            ALU.mult)
                    nc.vector.tensor_scalar(sgb[:, s], sgb[:, s], 1.0,
                                            LN_EPS * SCL, ALU.mult, ALU.add)
                    nc.vector.reciprocal(rc2b[:, s], sgb[:, s])
                    nc.vector.tensor_tensor(nbb[:, s], nmb[:, s],
                                            rc2b[:, s], ALU.mult)
                    bal.add("dve", 2600)

                # normalize + affine + store (round-robin engines so the
                # tail pipelines across all three)
                def d_pass3(qti):
                    yts = ytss[qti]
                    rec2, nb = rc2b[:, qti:qti + 1], nbb[:, qti:qti + 1]
                    o1 = p2.tile([128, D], F32, name=f"{nm}_o1{qti}",
                                 tag=f"o1{qti % 4}")
                    e = ("act", "dve", "pool")[qti % 3]
                    if e == "act":
                        nc.scalar.activation(o1[:], yts[:], AF.Identity,
                                             bias=nb, scale=rec2)
                    else:
                        bal.eng[e].tensor_scalar(o1[:], yts[:], rec2,
                                                 nb, ALU.mult, ALU.add)
                    if not ln_trivial:
                        e = ("dve", "pool")[qti % 2]
                        bal.eng[e].tensor_tensor(o1[:], o1[:], lna_t[:],
                                                 ALU.mult)
                        e = ("pool", "dve")[qti % 2]
                        bal.eng[e].tensor_tensor(o1[:], o1[:], lnb_t[:],
                                                 ALU.add)
                    nc.sync.dma_start(out_d[qti * 128:(qti + 1) * 128, :],
                                      o1[:])


                # ---- run phases; stage B of p+1 feeds into stage C of p ----
                load_phase(0)
                for mc in range(1, 4):
                    load_qt(mc)
                for g_ in proj_groups(0):
                    g_()
                pw_ts = []
                qres_ts = []
                for p in range(PHASES):
                    if p + 1 < PHASES:
                        load_phase(p + 1)
                        feeder = proj_groups(p + 1)
                    else:
                        feeder = iter(())
                        lna_t = p1.tile([128, D], F32, name=f"{nm}_lna")
                        lnb_t = p1.tile([128, D], F32, name=f"{nm}_lnb")
                        if not ln_trivial:
                            nc.sync.dma_start(
                                lna_t[:], lna_d[:].to_broadcast([128, D]))
                            nc.sync.dma_start(
                                lnb_t[:], lnb_d[:].to_broadcast([128, D]))
                        for oc in range(2):
                            pw_t = p2.tile([128, 8, 512], F8,
                                           name=f"{nm}_pw{oc}", tag="pw")
                            nc.sync.dma_start(
                                pw_t[:],
                                pw_d[oc * 128:(oc + 1) * 128, :].rearrange(
                                    "pp (dj f) -> pp dj f", dj=8))
                            pw_ts.append(pw_t)
                        for qti in range(8):
                            qr = pq.tile([128, D], F32,
                                         name=f"{nm}_qr{qti}", tag="qr")
                            nc.sync.dma_start(
                                qr[:], qres_d[qti * 128:(qti + 1) * 128, :])
                            qres_ts.append(qr)
                    if p + 1 < PHASES:
                        attention(p, feeder)
                    else:
                        late_feeder = iter(
                            [lambda qti=qti: d_pass1(qti, jmax=3)
                             for qti in range(4, 8)]
                            + [lambda qti=qti: d_pass1(qti)
                               for qti in range(4)]
                            + [lambda qti=qti: d_pass2(qti)
                               for qti in range(4)]
                            + [lambda: d_sigma_batch(0)]
                            + [lambda qti=qti: d_pass3(qti)
                               for qti in range(4)])
                        attention(p, feeder, late_feeder)


                # drain any leftover early passes, then finish tiles 4-7
                while True:
                    nxt = next(late_feeder, None)
                    if nxt is None:
                        break
                    nxt()
                for qti in range(4, 8):
                    d_pass1_fin(qti)
                for qti in range(4, 8):
                    d_pass2(qti)
                d_sigma_batch(4)
                for qti in range(4, 8):
                    d_pass3(qti)

    nc.compile()
    build.last_bal = bal.t
    return nc


def _get_nc(ln_trivial=False):
    key = ("nc", ln_trivial)
    if key not in _CACHE:
        _CACHE[key] = build(ln_trivial=ln_trivial)
    return _CACHE[key]


def prep_inputs(q, w_qs, w_ks, w_vs, proj_w, proj_b, ln_a, ln_b):
    f8 = ml_dtypes.float8_e4m3
    q = np.asarray(q, dtype=np.float32)
    w_qs = np.asarray(w_qs, dtype=np.float32)
    w_ks = np.asarray(w_ks, dtype=np.float32)
    w_vs = np.asarray(w_vs, dtype=np.float32)
    proj_w = np.asarray(proj_w, dtype=np.float32)
    proj_b = np.asarray(proj_b, dtype=np.float32)
    ln_a = np.asarray(ln_a, dtype=np.float32)
    ln_b = np.asarray(ln_b, dtype=np.float32)

    # dk-split column permutation for wq/wk: per phase the psum partitions
    # are [4 heads x dk 0:32 | 4 heads x dk 32:64]; dram layout is
    # partition-major: [p*128 + hf*... -> rows [P*128, cols dj*256+...]
    def perm_w(w):
        wt = (w * S_QK).transpose(1, 0, 2)  # [D, H, DK]
        # device tile [pp, dj, col]: col = hf*128 + hh*32 + j holds
        # head 4p+hh, dk hf*32+j, of d-row dj*128+pp.
        arr = np.empty((PHASES, 128, 8, 2, HP, 32), np.float32)
        for p in range(PHASES):
            for hh in range(HP):
                g = HP * p + hh
                w3 = wt[:, g, :].reshape(8, 128, DK)  # [dj, pp, dk]
                arr[p, :, :, 0, hh, :] = w3[:, :, 0:32].transpose(1, 0, 2)
                arr[p, :, :, 1, hh, :] = w3[:, :, 32:64].transpose(1, 0, 2)
        return np.ascontiguousarray(
            arr.reshape(PHASES * 128, 8 * 256)).astype(f8)

    wq_host = perm_w(w_qs)
    wk_host = perm_w(w_ks)

    wvt = (w_vs * S_V).transpose(1, 0, 2)  # [D, H, 64]
    wv_arr = np.zeros((PHASES, 128, 8, HP, 80), np.float32)
    for p in range(PHASES):
        for hh in range(HP):
            w3 = wvt[:, HP * p + hh, :].reshape(8, 128, 64)
            wv_arr[p, :, :, hh, 0:64] = w3.transpose(1, 0, 2)
    wv_host = np.ascontiguousarray(
        wv_arr.reshape(PHASES * 128, 8 * 320)).astype(f8)

    # pw: device tile per oc: [pp, dj, f=512]; rows hd = dj*128+pp
    pwt = (proj_w.T * S_P).reshape(8, 128, D)  # [dj, pp, out]
    pw_arr = np.empty((2, 128, 8, 512), np.float32)
    for oc in range(2):
        pw_arr[oc] = pwt[:, :, oc * 512:(oc + 1) * 512].transpose(1, 0, 2)
    pw_host = np.ascontiguousarray(pw_arr.reshape(256, 8 * 512)).astype(f8)

    lna = np.ascontiguousarray(ln_a[None, :])
    lnb = np.ascontiguousarray(ln_b[None, :])

    in_maps = []
    for c in range(8):
        b, half = c // 2, c % 2
        qbT = q[b].T  # [D, L]
        qt_c = np.concatenate(
            [qbT[:, half * HALF:(half + 1) * HALF],
             qbT[:, (1 - half) * HALF:(2 - half) * HALF]],
            axis=1,
        ).reshape(8, 128, L).transpose(1, 0, 2)  # [pp, dj, m]
        qt_c = np.ascontiguousarray(qt_c).astype(f8)
        qres_c = np.ascontiguousarray(
            (q[b, half * HALF:(half + 1) * HALF, :] + proj_b[None, :]) * SCL
        )
        in_maps.append({
            "qt": qt_c, "qres": qres_c,
            "wq": wq_host, "wk": wk_host, "wv": wv_host, "pw": pw_host,
            "lna": lna, "lnb": lnb,
        })
    return in_maps


def kernel(q, w_qs, w_ks, w_vs, proj_w, proj_b, ln_a, ln_b, **kw):
    in_maps = prep_inputs(q, w_qs, w_ks, w_vs, proj_w, proj_b, ln_a, ln_b)
    ln_trivial = bool(np.all(np.asarray(ln_a) == 1.0)
                      and np.all(np.asarray(ln_b) == 0.0))
    nc = _get_nc(ln_trivial)
    res = run_bass_kernel_spmd(nc, in_maps, core_ids=list(range(8))).results

    out = np.empty((B, L, D), dtype=np.float32)
    for c in range(8):
        b, half = c // 2, c % 2
        out[b, half * HALF:(half + 1) * HALF, :] = res[c]["out"]
    return out
